# revision 1
# baseline (speedup 1.0000x reference)
"""Sharded kNN (cosine retrieval) kernel for 8 Trainium2 NeuronCores.

Strategy
--------
Shard the memory bank (mem_descriptors, rank) across the 8 cores along N.
Host prep (cheap, O(N*F)): normalize rows of both matrices, zero out rows with
rank<=0, pad each 62500-row shard to 65536, transpose to [F, N_loc] and cast to
bf16 (the DMA/matmul-friendly layout).

Device (per core): score matrix S = x_hat^T-block @ y_hat tiles on the
TensorEngine (bf16 in, fp32 PSUM), fold the 128 PSUM tiles of each query block
into a [128, 2048] running-max accumulator on the VectorEngine (acc[j] = max
over columns n == j mod 2048), then one top-8 max + max_index per query block.
Device returns top-8 folded values + folded positions per (core, query-block).

Host finish (tiny): expand each near-max folded position into its 32 candidate
columns, rescore those few candidates exactly in fp32 against the original
data, and emit (cos_max, argmax index, gathered descriptor) exactly matching
the reference semantics.  Correct because the true winner's device score is
within ~2e-5 of the device max (bf16 rounding), far inside the 0.01 margin,
and every row's global max is positive (so zeroed invalid/padded entries never
win).
"""

import numpy as np
import ml_dtypes

M = 512          # queries
F = 256          # feature dim
N = 500000       # memory rows
NCORES = 8
N_PER = N // NCORES          # 62500
N_PAD = 65536                # per-core padded columns
ACC_W = 2048                 # folded accumulator width
N_BLOCKS = 16                # n-blocks of 4096 columns
BLK = 4096
PT = 512                     # psum tile width
MT = M // 128                # 4 query blocks
KT = F // 128                # 2 contraction tiles
MARGIN = 0.01

_CACHE = {}


def _build_bass():
    import concourse.mybir as mybir
    import concourse.tile as tile
    from concourse import bacc

    bf16 = mybir.dt.bfloat16
    f32 = mybir.dt.float32
    u32 = mybir.dt.uint32

    nc = bacc.Bacc("TRN2", target_bir_lowering=False, debug=False,
                   num_devices=NCORES)

    xt = nc.dram_tensor("xt", [F, M], bf16, kind="ExternalInput").ap()
    yt = nc.dram_tensor("yt", [F, N_PAD], bf16, kind="ExternalInput").ap()
    vals = nc.dram_tensor("vals", [M, 8], f32, kind="ExternalOutput").ap()
    idxs = nc.dram_tensor("idxs", [M, 8], u32, kind="ExternalOutput").ap()

    with tile.TileContext(nc) as tc:
        with (
            tc.tile_pool(name="xpool", bufs=1) as xpool,
            tc.tile_pool(name="ypool", bufs=3) as ypool,
            tc.tile_pool(name="apool", bufs=1) as apool,
            tc.tile_pool(name="opool", bufs=2) as opool,
            tc.tile_pool(name="psum", bufs=8, space="PSUM") as pspool,
        ):
            xts = []
            for k in range(KT):
                xk = xpool.tile([128, M], bf16, tag=f"x{k}")
                nc.sync.dma_start(out=xk[:], in_=xt[k * 128:(k + 1) * 128, :])
                xts.append(xk)

            accs = []
            for m in range(MT):
                a = apool.tile([128, ACC_W], f32, tag=f"acc{m}")
                nc.gpsimd.memset(a[:], -1e30)
                accs.append(a)

            for b in range(N_BLOCKS):
                ys = []
                for k in range(KT):
                    yk = ypool.tile([128, BLK], bf16, tag=f"y{k}")
                    nc.sync.dma_start(
                        out=yk[:],
                        in_=yt[k * 128:(k + 1) * 128, b * BLK:(b + 1) * BLK])
                    ys.append(yk)
                for m in range(MT):
                    for t in range(BLK // PT):
                        ps = pspool.tile([128, PT], f32, tag="ps")
                        nc.tensor.matmul(
                            ps[:], xts[0][:, m * 128:(m + 1) * 128],
                            ys[0][:, t * PT:(t + 1) * PT],
                            start=True, stop=False)
                        nc.tensor.matmul(
                            ps[:], xts[1][:, m * 128:(m + 1) * 128],
                            ys[1][:, t * PT:(t + 1) * PT],
                            start=False, stop=True)
                        sl = (b * (BLK // PT) + t) % (ACC_W // PT)
                        dst = accs[m][:, sl * PT:(sl + 1) * PT]
                        nc.vector.tensor_max(dst, dst, ps[:])

            for m in range(MT):
                t8 = opool.tile([128, 8], f32, tag="t8")
                i8 = opool.tile([128, 8], u32, tag="i8")
                nc.vector.max(t8[:], accs[m][:])
                nc.vector.max_index(i8[:], t8[:], accs[m][:])
                nc.sync.dma_start(out=vals[m * 128:(m + 1) * 128, :], in_=t8[:])
                nc.sync.dma_start(out=idxs[m * 128:(m + 1) * 128, :], in_=i8[:])

    nc.compile()
    return nc


def _get_nc():
    if "nc" not in _CACHE:
        _CACHE["nc"] = _build_bass()
    return _CACHE["nc"]


def run_device(in_maps, trace=False):
    """Run the compiled SPMD kernel; returns (results, BassKernelResults)."""
    from concourse.bass_utils import run_bass_kernel_spmd
    nc = _get_nc()
    r = run_bass_kernel_spmd(nc, in_maps, core_ids=list(range(NCORES)),
                             trace=trace)
    return r.results, r


def prep_inputs(descriptors, mem_descriptors, rank):
    """Host-side normalization / sharding / layout prep."""
    x = np.asarray(descriptors, np.float32)
    y = np.asarray(mem_descriptors, np.float32)
    r = np.asarray(rank, np.float32)

    xx = np.linalg.norm(x, axis=1, keepdims=True)          # [M,1]
    yy = np.linalg.norm(y, axis=1)                          # [N]
    xhat = (x / np.maximum(xx, 1e-30)).astype(np.float32)
    scale = np.where(r > 0, 1.0 / np.maximum(yy, 1e-30), 0.0).astype(np.float32)

    xt = np.ascontiguousarray(xhat.T).astype(ml_dtypes.bfloat16)  # [F, M]

    in_maps = []
    for c in range(NCORES):
        sh = y[c * N_PER:(c + 1) * N_PER]                   # [N_PER, F]
        sc = scale[c * N_PER:(c + 1) * N_PER]
        ytc = np.zeros((F, N_PAD), dtype=ml_dtypes.bfloat16)
        ytc[:, :N_PER] = (sh * sc[:, None]).T.astype(ml_dtypes.bfloat16)
        in_maps.append({"xt": xt, "yt": np.ascontiguousarray(ytc)})
    return in_maps, x, y, r, xx[:, 0], yy


def finish(results, x, y, r, xx, yy):
    """Expand device candidates, rescore exactly, emit reference-equal output."""
    vals = np.stack([np.asarray(results[c]["vals"], np.float32)
                     for c in range(NCORES)])               # [8, M, 8]
    idxs = np.stack([np.asarray(results[c]["idxs"], np.int64)
                     for c in range(NCORES)])               # [8, M, 8]

    gmax = vals.max(axis=(0, 2))                            # [M]
    keep = vals >= (gmax[None, :, None] - MARGIN)           # [8, M, 8]

    slices = np.arange(N_PAD // ACC_W) * ACC_W              # 32 fold slices

    cos_max = np.empty(M, np.float32)
    best_idx = np.empty(M, np.int64)
    valid = r > 0
    for m in range(M):
        cs, ss = np.nonzero(keep[:, m, :])
        cand = (idxs[cs, m, ss][:, None] + slices[None, :]).ravel()
        cores = np.repeat(cs, slices.size)
        ok = cand < N_PER
        ng = cores[ok] * N_PER + cand[ok]
        ng = np.unique(ng)
        ng = ng[valid[ng]]
        xy = y[ng] @ x[m]                                   # fp32 exact
        cos = xy / np.maximum(xx[m] * yy[ng], np.float32(1e-7))
        j = int(np.argmax(cos))
        ties = np.nonzero(cos == cos[j])[0]
        j = int(ties[np.argmin(ng[ties])])
        cos_max[m] = cos[j]
        best_idx[m] = ng[j]

    out_desc = y[best_idx]
    return (cos_max.astype(np.float32), best_idx.astype(np.int32),
            out_desc.astype(np.float32))


def kernel(descriptors, mem_descriptors, rank):
    in_maps, x, y, r, xx, yy = prep_inputs(descriptors, mem_descriptors, rank)
    results, _ = run_device(in_maps)
    return finish(results, x, y, r, xx, yy)


# revision 9
# speedup vs baseline: 1.3196x; 1.3196x over previous
"""Sharded kNN (cosine retrieval) kernel for 8 Trainium2 NeuronCores.

Strategy
--------
Shard the memory bank (mem_descriptors, rank) across the 8 cores along N.
Host prep (cheap, O(N*F)): normalize rows of both matrices, zero out rows with
rank<=0, pad each 62500-row shard to 65536, transpose to [F, N_loc] and cast to
bf16 (the DMA/matmul-friendly layout).

Device (per core): score matrix S = x_hat^T-block @ y_hat tiles on the
TensorEngine (bf16 in, fp32 PSUM), producing [128, 1024] two-bank PSUM groups.
Each group is max-folded into [128, 1024] running-max accumulators
(acc[j] = max over columns n == j mod 1024).  Fold work is split:
VectorE folds ~1/3 of the groups directly (fp32 from PSUM), ScalarE copies the
other ~2/3 to bf16 SBUF (with a +1.0 bias) and VectorE folds those at its 2x
bf16 rate.  One top-8 max + max_index per accumulator finishes the device
side.

Host finish (tiny): expand each near-max folded position into its candidate
columns, rescore those few candidates exactly in fp32 against the original
data, and emit (cos_max, argmax index, gathered descriptor) matching the
reference semantics.  Correct because the true winner's device score is within
~2e-3 of the device max (bf16 rounding), far inside the 0.01 margin, and every
row's global max is positive (so zeroed invalid/padded entries never win).
"""

import numpy as np
import ml_dtypes

M = 512          # queries
F = 256          # feature dim
N = 500000       # memory rows
NCORES = 8
N_PER = N // NCORES          # 62500
N_PAD = 65536                # per-core padded columns
ACC_W = 1024                 # folded accumulator width
N_BLOCKS = 16                # n-blocks of 4096 columns
BLK = 4096
GW = 1024                    # psum group width (2 banks)
PT = 512                     # single-bank matmul width
MT = M // 128                # 4 query blocks
KT = F // 128                # 2 contraction tiles
MARGIN = 0.01

# fold-engine pattern over psum groups:
# V = direct DVE fold (fp32 from PSUM), A = ACT copy (+bias, bf16) + DVE 2x fold
FOLD_PATTERN = ["A", "V", "A"]
BIAS = 1.0

_CACHE = {}


def _build_bass():
    import concourse.mybir as mybir
    import concourse.tile as tile
    from concourse import bacc

    bf16 = mybir.dt.bfloat16
    f32 = mybir.dt.float32
    u32 = mybir.dt.uint32

    nc = bacc.Bacc("TRN2", target_bir_lowering=False, debug=False,
                   num_devices=NCORES)

    xt = nc.dram_tensor("xt", [F, M], bf16, kind="ExternalInput").ap()
    yt = nc.dram_tensor("yt", [F, N_PAD], bf16, kind="ExternalInput").ap()
    # 16 = top-8 from the fp32 acc + top-8 from the bf16 acc
    vals = nc.dram_tensor("vals", [M, 16], f32, kind="ExternalOutput").ap()
    idxs = nc.dram_tensor("idxs", [M, 16], u32, kind="ExternalOutput").ap()

    with tile.TileContext(nc) as tc:
        with (
            tc.tile_pool(name="xpool", bufs=1) as xpool,
            tc.tile_pool(name="ypool", bufs=3) as ypool,
            tc.tile_pool(name="apool", bufs=1) as apool,
            tc.tile_pool(name="spool", bufs=4) as spool,
            tc.tile_pool(name="opool", bufs=4) as opool,
            tc.tile_pool(name="psum", bufs=4, space="PSUM") as pspool,
        ):
            xts = []
            for k in range(KT):
                xk = xpool.tile([128, M], bf16, tag=f"x{k}")
                nc.sync.dma_start(out=xk[:], in_=xt[k * 128:(k + 1) * 128, :])
                xts.append(xk)

            acc_v, acc_g = [], []
            for m in range(MT):
                av = apool.tile([128, ACC_W], f32, tag=f"accv{m}")
                nc.gpsimd.memset(av[:], -1e30)
                acc_v.append(av)
                ag = apool.tile([128, ACC_W], bf16, tag=f"accg{m}")
                nc.gpsimd.memset(ag[:], 0.0)   # biased scores are all >= 0.7
                acc_g.append(ag)

            gg = 0  # global psum-group counter, drives the fold pattern
            for b in range(N_BLOCKS):
                ys = []
                for k in range(KT):
                    yk = ypool.tile([128, BLK], bf16, tag=f"y{k}")
                    nc.sync.dma_start(
                        out=yk[:],
                        in_=yt[k * 128:(k + 1) * 128, b * BLK:(b + 1) * BLK])
                    ys.append(yk)
                for m in range(MT):
                    for grp in range(BLK // GW):
                        ps = pspool.tile([128, GW], f32, tag="ps")
                        for k in range(KT):
                            for sub in range(GW // PT):
                                c0 = grp * GW + sub * PT
                                nc.tensor.matmul(
                                    ps[:, sub * PT:(sub + 1) * PT],
                                    xts[k][:, m * 128:(m + 1) * 128],
                                    ys[k][:, c0:c0 + PT],
                                    start=(k == 0), stop=(k == KT - 1))
                        if FOLD_PATTERN[gg % len(FOLD_PATTERN)] == "V":
                            dst = acc_v[m][:]
                            nc.vector.tensor_max(dst, dst, ps[:])
                        else:
                            stg = spool.tile([128, GW], bf16, tag="stg")
                            nc.scalar.add(stg[:], ps[:], BIAS)
                            dst = acc_g[m][:]
                            nc.vector.tensor_max(dst, dst, stg[:])
                        gg += 1

            for m in range(MT):
                t8 = opool.tile([128, 8], f32, tag="t8")
                i8 = opool.tile([128, 8], u32, tag="i8")
                nc.vector.max(t8[:], acc_v[m][:])
                nc.vector.max_index(i8[:], t8[:], acc_v[m][:])
                nc.sync.dma_start(out=vals[m * 128:(m + 1) * 128, 0:8],
                                  in_=t8[:])
                nc.sync.dma_start(out=idxs[m * 128:(m + 1) * 128, 0:8],
                                  in_=i8[:])

                t8g = opool.tile([128, 8], bf16, tag="t8g")
                t8gf = opool.tile([128, 8], f32, tag="t8gf")
                i8g = opool.tile([128, 8], u32, tag="i8g")
                nc.vector.max(t8g[:], acc_g[m][:])
                nc.vector.max_index(i8g[:], t8g[:], acc_g[m][:])
                # un-bias while converting bf16 -> f32
                nc.vector.tensor_scalar_add(t8gf[:], t8g[:], -BIAS)
                nc.sync.dma_start(out=vals[m * 128:(m + 1) * 128, 8:16],
                                  in_=t8gf[:])
                nc.sync.dma_start(out=idxs[m * 128:(m + 1) * 128, 8:16],
                                  in_=i8g[:])

    nc.compile()
    return nc


def _get_nc():
    if "nc" not in _CACHE:
        _CACHE["nc"] = _build_bass()
    return _CACHE["nc"]


def run_device(in_maps, trace=False):
    """Run the compiled SPMD kernel; returns (results, BassKernelResults)."""
    from concourse.bass_utils import run_bass_kernel_spmd
    nc = _get_nc()
    r = run_bass_kernel_spmd(nc, in_maps, core_ids=list(range(NCORES)),
                             trace=trace)
    return r.results, r


def prep_inputs(descriptors, mem_descriptors, rank):
    """Host-side normalization / sharding / layout prep."""
    x = np.asarray(descriptors, np.float32)
    y = np.asarray(mem_descriptors, np.float32)
    r = np.asarray(rank, np.float32)

    xx = np.linalg.norm(x, axis=1, keepdims=True)          # [M,1]
    yy = np.linalg.norm(y, axis=1)                          # [N]
    xhat = (x / np.maximum(xx, 1e-30)).astype(np.float32)
    scale = np.where(r > 0, 1.0 / np.maximum(yy, 1e-30), 0.0).astype(np.float32)

    xt = np.ascontiguousarray(xhat.T).astype(ml_dtypes.bfloat16)  # [F, M]

    in_maps = []
    for c in range(NCORES):
        sh = y[c * N_PER:(c + 1) * N_PER]                   # [N_PER, F]
        sc = scale[c * N_PER:(c + 1) * N_PER]
        ytc = np.zeros((F, N_PAD), dtype=ml_dtypes.bfloat16)
        ytc[:, :N_PER] = (sh * sc[:, None]).T.astype(ml_dtypes.bfloat16)
        in_maps.append({"xt": xt, "yt": np.ascontiguousarray(ytc)})
    return in_maps, x, y, r, xx[:, 0], yy


def finish(results, x, y, r, xx, yy):
    """Expand device candidates, rescore exactly, emit reference-equal output."""
    vals = np.stack([np.asarray(results[c]["vals"], np.float32)
                     for c in range(NCORES)])               # [8, M, 16]
    idxs = np.stack([np.asarray(results[c]["idxs"], np.int64)
                     for c in range(NCORES)])               # [8, M, 16]

    gmax = vals.max(axis=(0, 2))                            # [M]
    keep = vals >= (gmax[None, :, None] - MARGIN)           # [8, M, 16]

    slices = np.arange(N_PAD // ACC_W) * ACC_W              # fold slices

    cos_max = np.empty(M, np.float32)
    best_idx = np.empty(M, np.int64)
    valid = r > 0
    for m in range(M):
        cs, ss = np.nonzero(keep[:, m, :])
        cand = (idxs[cs, m, ss][:, None] + slices[None, :]).ravel()
        cores = np.repeat(cs, slices.size)
        ok = cand < N_PER
        ng = cores[ok] * N_PER + cand[ok]
        ng = np.unique(ng)
        ng = ng[valid[ng]]
        xy = y[ng] @ x[m]                                   # fp32 exact
        cos = xy / np.maximum(xx[m] * yy[ng], np.float32(1e-7))
        j = int(np.argmax(cos))
        ties = np.nonzero(cos == cos[j])[0]
        j = int(ties[np.argmin(ng[ties])])
        cos_max[m] = cos[j]
        best_idx[m] = ng[j]

    out_desc = y[best_idx]
    return (cos_max.astype(np.float32), best_idx.astype(np.int32),
            out_desc.astype(np.float32))


def kernel(descriptors, mem_descriptors, rank):
    in_maps, x, y, r, xx, yy = prep_inputs(descriptors, mem_descriptors, rank)
    results, _ = run_device(in_maps)
    return finish(results, x, y, r, xx, yy)
